# revision 1
# baseline (speedup 1.0000x reference)
"""EpisodicMemory Trainium2 kernel, v2.

Data-parallel over batch across 8 NeuronCores (128 batch rows per core).

Layout is "flipped": the GRU state h and all gate pre-activations live as
[H-on-partitions (4 k-tiles of 128), batch-on-free(128)] tiles, so the
recurrent matmul h @ W_hh^T needs NO transposes: its rhs (moving operand)
is h itself, and the elementwise update produces h directly in that
layout. Per-sentence episodic gates are broadcast across partitions with a
stride-0 DMA from DRAM.

Precision plan (validated offline, rel err ~3e-3 vs 2e-2 budget):
 - h state, gates r/w/n/a and W_hh: float16 (DVE 2x mode, matmul 1 cyc/row)
 - x-part (C @ W_ih^T) and scoring fc1: 3-term error-compensated fp8-e4m3
   DoubleRow matmuls (hi/lo splits of both operands, Whi@Chi + Whi@Clo +
   Wlo@Chi), 0.5 cyc/row with K=256 per instruction = 4x fp32r throughput
   per term.
 - All W matrices pre-scaled so every GRU psum holds 512x the preact
   (fc1: 128x); the 1/512 fold into the ACT sigmoid/tanh scale.
 - C / feat = [C*Q, C*prev_M, |C-Q|, |C-prev_M|] quantized host-side.
"""
import numpy as np
import ml_dtypes

H = 512
SH = 120
B = 1024
S = 64
NCORES = 8
BPC = B // NCORES  # 128
KH = H // 128      # 4
G3 = 3 * H
SGRP = 4
NGRP = S // SGRP   # 16
NT = G3 // 128     # 12 gate tiles
FK = 4 * H // 128  # 16 feat k-tiles
E4 = ml_dtypes.float8_e4m3
F16 = np.float16

XMODE = "fp8x3"    # "fp16" or "fp8x3"  (x-part matmul mode)
F1MODE = "fp8x3"   # fc1 matmul mode

_CACHE = {}


def _q8(x):
    return np.clip(np.asarray(x, np.float32), -240.0, 240.0).astype(E4)


def _split8(x, scale):
    hi = _q8(x * scale)
    lo = _q8(x * scale - hi.astype(np.float32))
    return hi, lo


def _pairs(w, scale):
    """[G, K] weight -> DR lhsT hi/lo [128, K//256, 2, G] fp8."""
    hi, lo = _split8(np.ascontiguousarray(w.T), scale)  # [K, G]
    K = w.shape[1]
    out = []
    for t in (hi, lo):
        out.append(np.ascontiguousarray(
            t.reshape(K // 256, 2, 128, w.shape[0]).transpose(2, 0, 1, 3)))
    return out  # each [128, K//256, 2, G]


def _build(consts):
    import concourse.bass as bass
    import concourse.tile as tile
    from concourse import bacc, mybir

    FP32 = mybir.dt.float32
    FP16 = mybir.dt.float16
    FP8 = mybir.dt.float8e4
    OP = mybir.AluOpType
    AF = mybir.ActivationFunctionType
    PM = mybir.MatmulPerfMode

    nc = bacc.Bacc("TRN2", target_bir_lowering=False, debug=False,
                   num_devices=NCORES)

    # ---- external inputs (per core) ----
    if XMODE == "fp16":
        c_t = nc.dram_tensor("c16", [S, 128, KH, BPC], FP16,
                             kind="ExternalInput")
    else:
        c_t = nc.dram_tensor("c8", [2, S, 128, 2, 2, BPC], FP8,
                             kind="ExternalInput")  # [hi/lo, s, p, pair, i, b]
    if F1MODE == "fp16":
        f_t = nc.dram_tensor("feat16", [NGRP, 128, FK, SGRP * BPC], FP16,
                             kind="ExternalInput")
    else:
        f_t = nc.dram_tensor("feat8", [2, NGRP, 128, FK // 2, 2, SGRP * BPC],
                             FP8, kind="ExternalInput")
    out = nc.dram_tensor("out", [BPC, H], FP32, kind="ExternalOutput")

    # ---- inline consts ----
    dl = {}
    for k, v in consts.items():
        dl[k] = nc.inline_tensor(v, name=k)

    from contextlib import ExitStack
    with tile.TileContext(nc) as tc:
        with ExitStack() as ctx:
            cpool = ctx.enter_context(tc.tile_pool(name="const", bufs=1))
            cpool2 = ctx.enter_context(tc.tile_pool(name="const2", bufs=1))
            cstr = ctx.enter_context(tc.tile_pool(name="cstr", bufs=6))
            fstr = ctx.enter_context(tc.tile_pool(name="fstr", bufs=3))
            hpool = ctx.enter_context(tc.tile_pool(name="h", bufs=3))
            gpool = ctx.enter_context(tc.tile_pool(name="g", bufs=4))
            gdram = ctx.enter_context(tc.tile_pool(name="gd", bufs=4,
                                                   space="DRAM"))
            ew = ctx.enter_context(tc.tile_pool(name="ew", bufs=3))
            ew2 = ctx.enter_context(tc.tile_pool(name="ew2", bufs=3))
            sco = ctx.enter_context(tc.tile_pool(name="sco", bufs=2))
            ps_r = ctx.enter_context(tc.tile_pool(name="ps_r", bufs=2,
                                                  space="PSUM"))
            ps_z = ctx.enter_context(tc.tile_pool(name="ps_z", bufs=2,
                                                  space="PSUM"))
            ps_x = ctx.enter_context(tc.tile_pool(name="ps_x", bufs=2,
                                                  space="PSUM"))
            ps_h = ctx.enter_context(tc.tile_pool(name="ps_h", bufs=1,
                                                  space="PSUM"))
            ps_f = ctx.enter_context(tc.tile_pool(name="ps_f", bufs=1,
                                                  space="PSUM"))

            # ---- load consts ----
            whh = cpool.tile([128, KH, G3], FP16, tag="whh")
            nc.sync.dma_start(whh[:], dl["whh16"].ap())
            idt = cpool.tile([128, 128], FP32, tag="idt")
            nc.sync.dma_start(idt[:], dl["ident"].ap())
            if XMODE == "fp16":
                wih = cpool.tile([128, KH, G3], FP16, tag="wih")
                nc.sync.dma_start(wih[:], dl["wih16"].ap())
            else:
                # [p, pair, gate-tile, i, 128]: each [:, pair, jj] slice is a
                # CONTIGUOUS [128, 2, 128] block (walrus dual-fp8 ldweights
                # rejects strided weight APs)
                wih_h = cpool.tile([128, 2, NT, 2, 128], FP8, tag="wih_h")
                nc.sync.dma_start(wih_h[:], dl["wih8h"].ap())
                wih_l = cpool.tile([128, 2, NT, 2, 128], FP8, tag="wih_l")
                nc.sync.dma_start(wih_l[:], dl["wih8l"].ap())
            if F1MODE == "fp16":
                f1t = cpool2.tile([128, FK, SH], FP16, tag="f1t")
                nc.sync.dma_start(f1t[:], dl["f1t16"].ap())
            else:
                # SH padded to 128: standalone dual-fp8 ldweights rejects
                # M=120 (s3_lw_dual_fp8_restrictions)
                f1h = cpool2.tile([128, FK // 2, 2, 128], FP8, tag="f1h")
                nc.sync.dma_start(f1h[:], dl["f18h"].ap())
                f1l = cpool2.tile([128, FK // 2, 2, 128], FP8, tag="f1l")
                nc.sync.dma_start(f1l[:], dl["f18l"].ap())
            f2t = cpool2.tile([SH, 1], FP16, tag="f2t")
            nc.sync.dma_start(f2t[:], dl["f2t16"].ap())
            # touch every activation function once so the ACT table loads
            # happen during the const-DMA wait, not on the scan chain
            warm = cpool2.tile([1, 4], FP32, tag="warm")
            for af in (AF.Sigmoid, AF.Tanh, AF.Copy):
                nc.scalar.activation(warm[:], idt[0:1, 0:4], af)

            # ================= helpers =================
            def load_c(s):
                if XMODE == "fp16":
                    ct = cstr.tile([128, KH, BPC], FP16, tag="cs")
                    nc.sync.dma_start(ct[:], c_t.ap()[s])
                    return (ct,)
                ch = cstr.tile([128, 2, 2, BPC], FP8, tag="csh")
                nc.sync.dma_start(ch[:], c_t.ap()[0, s])
                cl = cstr.tile([128, 2, 2, BPC], FP8, tag="csl")
                nc.sync.dma_start(cl[:], c_t.ap()[1, s])
                return (ch, cl)

            def x_mms(cts, banks, with_stop):
                """x-part matmuls for one sentence into the given
                [(psum, gate)] banks. start on each bank's first matmul,
                stop on its last iff with_stop."""
                if XMODE == "fp16":
                    (ct,) = cts
                    for pb, gate in banks:
                        for jj in range(4):
                            g0 = gate * H + jj * 128
                            for kt in range(KH):
                                nc.tensor.matmul(
                                    pb[:, jj], wih[:, kt, g0:g0 + 128],
                                    ct[:, kt],
                                    start=(jj == 0 and kt == 0),
                                    stop=(with_stop and jj == 3
                                          and kt == KH - 1))
                else:
                    ch, cl = cts
                    for pb, gate in banks:
                        for jj in range(4):
                            gt_ = gate * 4 + jj
                            for pair in range(2):
                                terms = [(wih_h, ch), (wih_h, cl),
                                         (wih_l, ch)]
                                for ti, (wt_, ct_) in enumerate(terms):
                                    nc.tensor.matmul(
                                        pb[:, jj],
                                        wt_[:, pair, gt_],
                                        ct_[:, pair],
                                        start=(jj == 0 and pair == 0
                                               and ti == 0),
                                        stop=(with_stop and jj == 3
                                              and pair == 1 and ti == 2),
                                        perf_mode=PM.DoubleRow)

            def h_mms(h16, pr, pz, pxn, phn):
                """h-part matmuls: r tiles, n tiles, z tiles."""
                for pb, gate in [(pr, 0), (pxn, 2), (pz, 1)]:
                    tgt = phn if gate == 2 else pb
                    for jj in range(4):
                        g0 = gate * H + jj * 128
                        for kt in (2, 3, 0, 1):
                            nc.tensor.matmul(
                                tgt[:, jj], whh[:, kt, g0:g0 + 128],
                                h16[:, kt],
                                start=(gate == 2 and jj == 0 and kt == 2),
                                stop=(jj == 3 and kt == 1))

            # ---- scoring machinery ----
            grp = {}

            def load_feat(gi):
                # split the load in halves along the k dim so downstream
                # fc1 chunks unblock progressively and queues spread
                if F1MODE == "fp16":
                    ft = fstr.tile([128, FK, SGRP * BPC], FP16, tag="feat")
                    nc.sync.dma_start(ft[:, 0:FK // 2],
                                      f_t.ap()[gi, :, 0:FK // 2])
                    nc.sync.dma_start(ft[:, FK // 2:FK],
                                      f_t.ap()[gi, :, FK // 2:FK])
                    grp[gi] = {"feat": (ft,)}
                else:
                    fh = fstr.tile([128, FK // 2, 2, SGRP * BPC], FP8,
                                   tag="feath")
                    nc.sync.dma_start(fh[:, 0:4], f_t.ap()[0, gi, :, 0:4])
                    nc.sync.dma_start(fh[:, 4:8], f_t.ap()[0, gi, :, 4:8])
                    fl = fstr.tile([128, FK // 2, 2, SGRP * BPC], FP8,
                                   tag="featl")
                    nc.sync.dma_start(fl[:, 0:4], f_t.ap()[1, gi, :, 0:4])
                    nc.sync.dma_start(fl[:, 4:8], f_t.ap()[1, gi, :, 4:8])
                    grp[gi] = {"feat": (fh, fl)}

            def fc1_chunk(gi, q):
                """Quarter q of group gi's fc1 matmuls."""
                st = grp[gi]
                if q == 0:
                    np_ = 128 if F1MODE == "fp8x3" else SH
                    st["pps"] = ps_f.tile([np_, SGRP * BPC], FP32, tag="pps",
                                          name="pps")
                pps = st["pps"]
                if F1MODE == "fp16":
                    (ft,) = st["feat"]
                    for kt in range(4 * q, 4 * q + 4):
                        nc.tensor.matmul(pps[:], f1t[:, kt], ft[:, kt],
                                         start=(kt == 0), stop=(kt == FK - 1))
                else:
                    fh, fl = st["feat"]
                    for pair in range(2 * q, 2 * q + 2):
                        terms = [(f1h, fh), (f1h, fl), (f1l, fh)]
                        for ti, (wt_, ft_) in enumerate(terms):
                            nc.tensor.matmul(
                                pps[:], wt_[:, pair], ft_[:, pair],
                                start=(pair == 0 and ti == 0),
                                stop=(pair == FK // 2 - 1 and ti == 2),
                                perf_mode=PM.DoubleRow)
                if q == 3:
                    finish_group(gi)

            def finish_group(gi):
                st = grp[gi]
                pps = st["pps"]
                h1 = sco.tile([SH, SGRP * BPC], FP16, tag="h1")
                nc.scalar.activation(h1[:], pps[0:SH, :], AF.Tanh,
                                     scale=1.0 / 128)
                nc.tensor.matmul(pps[0:1, :], f2t[:], h1[:],
                                 start=True, stop=True)
                gt = gpool.tile([1, SGRP * BPC], FP16, tag="gt")
                nc.scalar.activation(gt[:], pps[0:1, :], AF.Sigmoid)
                gd = gdram.tile([1, SGRP * BPC], FP16, tag="gd")
                nc.sync.dma_start(gd[:], gt[:])
                grep = gpool.tile([128, SGRP, BPC], FP16, tag="grep")
                nc.sync.dma_start(
                    grep[:], gd[:].broadcast_to([128, SGRP * BPC]))
                st["grep"] = grep
                del st["pps"], st["feat"]

            # ================= prologue =================
            for gi in (0, 1, 2):
                load_feat(gi)
            for gi in (0, 1):
                for q in range(4):
                    fc1_chunk(gi, q)
            cbuf = {}
            for s in range(0, 5):
                cbuf[s] = load_c(s)

            # x matmuls for s=0 (no r needed: h=0 -> r*hn=0)
            pz = ps_z.tile([128, 4, BPC], FP32, tag="pz", name="pz")
            pxn = ps_x.tile([128, 4, BPC], FP32, tag="pxn", name="pxn")
            x_mms(cbuf[0], [(pxn, 2), (pz, 1)], with_stop=True)
            del cbuf[0]

            h16 = None
            pr = None
            ISC = 1.0 / 512

            # ================= scan =================
            for s in range(S):
                # ---- h-part matmuls (s>0) ----
                if s > 0:
                    phn = ps_h.tile([128, 4, BPC], FP32, tag="phn",
                                    name="phn")
                    h_mms(h16, pr, pz, pxn, phn)

                # ---- elementwise chain ----
                gi = s // SGRP
                j = s % SGRP
                grep = grp[gi]["grep"]

                if s > 0:
                    r_sb = ew.tile([128, 4, BPC], FP16, tag="r")
                    nc.scalar.activation(r_sb[:], pr[:], AF.Sigmoid,
                                         scale=ISC)
                    tn = ew2.tile([128, 4, BPC], FP32, tag="tn")
                    nc.vector.tensor_tensor(tn[:], r_sb[:], phn[:], OP.mult)
                w_sb = ew.tile([128, 4, BPC], FP16, tag="w")
                nc.scalar.activation(w_sb[:], pz[:], AF.Sigmoid, scale=-ISC)
                if s > 0:
                    tn2 = ew2.tile([128, 4, BPC], FP32, tag="tn2")
                    nc.vector.tensor_tensor(tn2[:], tn[:], pxn[:], OP.add)
                a_sb = ew.tile([128, 4, BPC], FP16, tag="a")
                nc.vector.tensor_tensor(
                    a_sb[:], w_sb[:],
                    grep[:, j].unsqueeze(1).broadcast_to([128, 4, BPC]),
                    OP.mult)
                # p = (1-a)*h, computed OFF the critical chain (a and h are
                # both ready before tanh) so the post-tanh tail is only
                # m2 = a*n ; h' = p + m2  (2 hops instead of 3)
                if s > 0:
                    t1 = ew2.tile([128, 4, BPC], FP16, tag="t1")
                    nc.vector.tensor_tensor(t1[:], a_sb[:], h16[:], OP.mult)
                    p_sb = ew2.tile([128, 4, BPC], FP16, tag="p")
                    nc.vector.tensor_tensor(p_sb[:], h16[:], t1[:],
                                            OP.subtract)
                n_sb = ew.tile([128, 4, BPC], FP16, tag="n")
                if s > 0:
                    nc.scalar.activation(n_sb[:], tn2[:], AF.Tanh, scale=ISC)
                else:
                    nc.scalar.activation(n_sb[:], pxn[:], AF.Tanh, scale=ISC)

                last = s == S - 1
                if last:
                    nh = ew.tile([128, KH, BPC], FP32, tag="hf")
                else:
                    nh = hpool.tile([128, KH, BPC], FP16, tag="h", name="h")
                if s == 0:
                    for half in (1, 0):
                        k0 = 2 * half
                        nc.vector.tensor_tensor(
                            nh[:, k0:k0 + 2], a_sb[:, k0:k0 + 2],
                            n_sb[:, k0:k0 + 2], OP.mult)
                else:
                    for half in (1, 0):
                        k0 = 2 * half
                        m_h = ew2.tile([128, 2, BPC], FP16, tag=f"m{half}",
                                       name=f"m{half}")
                        nc.vector.tensor_tensor(
                            m_h[:], a_sb[:, k0:k0 + 2], n_sb[:, k0:k0 + 2],
                            OP.mult)
                        nc.vector.tensor_tensor(
                            nh[:, k0:k0 + 2], p_sb[:, k0:k0 + 2], m_h[:],
                            OP.add)
                h16 = nh

                # ---- x-part matmuls for s+1 ----
                if s + 1 < S:
                    pr = ps_r.tile([128, 4, BPC], FP32, tag="pr", name="pr")
                    pz = ps_z.tile([128, 4, BPC], FP32, tag="pz", name="pz")
                    pxn = ps_x.tile([128, 4, BPC], FP32, tag="pxn",
                                    name="pxn")
                    x_mms(cbuf[s + 1], [(pr, 0), (pxn, 2), (pz, 1)],
                          with_stop=False)
                    del cbuf[s + 1]
                    if s + 5 < S:
                        cbuf[s + 5] = load_c(s + 5)

                # ---- interleaved scoring ----
                gisc = s // SGRP + 2
                q = s % SGRP
                if gisc <= NGRP - 1:
                    if q == 0 and gisc + 1 <= NGRP - 1:
                        load_feat(gisc + 1)
                    fc1_chunk(gisc, q)

            # ================= epilogue =================
            pt = ps_r.tile([128, 4, BPC], FP32, tag="pr", name="ptr")
            for kt in range(KH):
                nc.tensor.transpose(pt[:, kt], h16[:, kt], idt[:])
            ot = ew.tile([128, H], FP32, tag="ot")
            nc.scalar.activation(ot[:], pt[:].rearrange("p k b -> p (k b)"),
                                 AF.Copy)
            nc.sync.dma_start(out.ap(), ot[:])

    nc.compile()
    return nc


def _prep(C, Q, prev_M, fc1_w, fc2_w, W_ih, W_hh):
    """Host-side layout/dtype transforms + per-core sharding."""
    consts = {}
    # W_hh^T * 512 as fp16, [128, KH, G3] with h = kt*128 + p
    whhT = np.ascontiguousarray(W_hh.T * 512.0)  # [H, G3]
    consts["whh16"] = np.ascontiguousarray(
        whhT.reshape(KH, 128, G3).transpose(1, 0, 2)).astype(F16)
    consts["ident"] = np.eye(128, dtype=np.float32)
    if XMODE == "fp16":
        wihT = np.ascontiguousarray(W_ih.T * 512.0)
        consts["wih16"] = np.ascontiguousarray(
            wihT.reshape(KH, 128, G3).transpose(1, 0, 2)).astype(F16)
    else:
        hi, lo = _split8(np.ascontiguousarray(W_ih.T), 16.0)  # [K, G3]
        for nm, t in (("wih8h", hi), ("wih8l", lo)):
            consts[nm] = np.ascontiguousarray(
                t.reshape(2, 2, 128, NT, 128).transpose(2, 0, 3, 1, 4))
    if F1MODE == "fp16":
        f1T = np.ascontiguousarray(fc1_w.T * 128.0)
        consts["f1t16"] = np.ascontiguousarray(
            f1T.reshape(FK, 128, SH).transpose(1, 0, 2)).astype(F16)
    else:
        f1p = np.zeros((128, 4 * H), np.float32)  # pad SH 120 -> 128
        f1p[:SH] = fc1_w
        consts["f18h"], consts["f18l"] = _pairs(f1p, 16.0)
    consts["f2t16"] = np.ascontiguousarray(fc2_w.T).astype(F16)

    in_maps = []
    for c in range(NCORES):
        lo, hi = c * BPC, (c + 1) * BPC
        Cc = np.ascontiguousarray(C[lo:hi])          # [BPC, S, H]
        Qc = Q[lo:hi, 0]                              # [BPC, H]
        Mc = prev_M[lo:hi, 0]
        m = {}
        # C in [s, p, kt, b] (fp16) or fp8 pair layout
        Ct = np.ascontiguousarray(Cc.transpose(1, 2, 0))   # [S, H, BPC]
        if XMODE == "fp16":
            m["c16"] = np.ascontiguousarray(
                Ct.reshape(S, KH, 128, BPC).transpose(0, 2, 1, 3)
            ).astype(F16)
        else:
            chi, clo = _split8(Ct, 32.0)  # [S, H, BPC]
            both = np.stack([chi, clo])   # [2, S, H, BPC]
            m["c8"] = np.ascontiguousarray(
                both.reshape(2, S, 2, 2, 128, BPC).transpose(0, 1, 4, 2, 3, 5))
        # feat [BPC, S, 4H]
        feat = np.concatenate(
            [Cc * Qc[:, None, :], Cc * Mc[:, None, :],
             np.abs(Cc - Qc[:, None, :]), np.abs(Cc - Mc[:, None, :])],
            axis=2)
        # -> [NGRP, 4H, SGRP, BPC]
        ftr = np.ascontiguousarray(
            feat.transpose(1, 2, 0).reshape(NGRP, SGRP, 4 * H, BPC)
            .transpose(0, 2, 1, 3))
        if F1MODE == "fp16":
            m["feat16"] = np.ascontiguousarray(
                ftr.reshape(NGRP, FK, 128, SGRP * BPC)
                .transpose(0, 2, 1, 3)).astype(F16)
        else:
            fhi, flo = _split8(ftr, 8.0)       # [NGRP, 4H, SGRP, BPC]
            both = np.stack([fhi, flo])
            m["feat8"] = np.ascontiguousarray(
                both.reshape(2, NGRP, FK // 2, 2, 128, SGRP * BPC)
                .transpose(0, 1, 4, 2, 3, 5))
        in_maps.append(m)
    return consts, in_maps


def kernel(C, Q, prev_M, fc1_w, fc1_b, fc2_w, fc2_b, W_ih, W_hh, b_ih, b_hh):
    from concourse.bass_utils import run_bass_kernel_spmd

    C = np.asarray(C, dtype=np.float32)
    Q = np.asarray(Q, dtype=np.float32)
    prev_M = np.asarray(prev_M, dtype=np.float32)
    fc1_w = np.asarray(fc1_w, np.float32)
    fc2_w = np.asarray(fc2_w, np.float32)
    W_ih = np.asarray(W_ih, np.float32)
    W_hh = np.asarray(W_hh, np.float32)
    fc1_b = np.asarray(fc1_b, np.float32)
    fc2_b = np.asarray(fc2_b, np.float32)
    b_ih = np.asarray(b_ih, np.float32)
    b_hh = np.asarray(b_hh, np.float32)
    assert not (np.any(fc1_b) or np.any(fc2_b) or np.any(b_ih)
                or np.any(b_hh)), "nonzero biases unsupported in v2"

    consts, in_maps = _prep(C, Q, prev_M, fc1_w, fc2_w, W_ih, W_hh)

    key = tuple(np.asarray(v).tobytes() for v in consts.values())
    kh = hash(key)
    if kh not in _CACHE:
        _CACHE[kh] = _build(consts)
    nc = _CACHE[kh]

    res = run_bass_kernel_spmd(nc, in_maps, list(range(NCORES)))
    h = np.concatenate([res.results[c]["out"] for c in range(NCORES)],
                       axis=0)
    return h[:, None, :].astype(np.float32)

